# revision 11
# baseline (speedup 1.0000x reference)
"""Multi-head attention (N=4096, D=1024, 16 heads) on 8 trn2 NeuronCores.

Sharding: tensor-parallel over heads. Each core owns 2 heads (128 of the
1024 QKV projection columns / W_o rows), computes its heads' attention
fully on-device, applies its W_o row-slice, and returns a partial
[4096, 1024] output. The host sums the 8 partials (the "all-reduce").

Device kernel per core (fp16 matmuls, fp32 PSUM accumulation):
  1. QT/KT/VT = (W^T x^T) chunks [128 = 2*64 head dims, 512 cols]; V is
     PE-transposed to row-major [row 128, V0 | 1 | V1 | 1]. K and Q get
     partition-swapped duplicates (via SBUF->SBUF DMA) so each head's
     K/Q slice exists on BOTH partition halves.
  2. Attention sweeps 1024 queries (isup) x head x key-tile-PAIR. The
     two score matmul groups of a pair run on opposite PE quadrants
     (rows 0-63 vs 64-127, via the duplicate tiles) and the hardware
     executes them CONCURRENTLY (~2x on scores). exp() is split between
     ScalarE (activation table) and DVE (Schraudolph int16 bit-trick,
     zero-mean calibrated) to keep both under the PE roofline.
  3. u accumulates in PSUM [65, 1024] (V|1 rows); at head seams u is
     drained, normalized (broadcast-matmul + reciprocal + multiply) into
     a head-stacked [128, 1024] tile (h1 partition-shifted by DMA), and
     projected with ONE contraction-128 matmul per output tile.
"""

import numpy as np
import ml_dtypes  # noqa: F401

import concourse.bass as bass  # noqa: F401
import concourse.tile as tile
from concourse import bacc, mybir
from concourse.bass_utils import run_bass_kernel_spmd
import concourse.alu_op_type as alu

F16 = mybir.dt.float16
F32 = mybir.dt.float32
I16 = mybir.dt.int16
EXP = mybir.ActivationFunctionType.Exp

N = 4096
DIN = 1024
DOUT = 1024
NCORES = 8
DPC = 128  # dims per core = 2 heads * 64
HD = 64

# Schraudolph exp on DVE: int16(round(s*SCH_A + SCH_B)) bits viewed as
# fp16 == exp(s/8) * (1 + eps), eps zero-mean, rms 1.8%, |eps| < 4%.
SCH_A = 0.125 / float(np.log(2.0)) * 1024.0
SCH_B = 15360.0 - 59.0
DVE_EXP_MOD = 6  # every 6th exp unit goes to DVE (alpha = 1/6)


def emit(tc, outs, ins, n, din):
    nc = tc.nc
    qT, kT, vT, wq, wk, wv, wo, ident = ins
    out = outs[0]

    nkt = din // 128          # contraction tiles for projections
    nch = n // 512            # 512-wide column chunks of QT/KT/VT
    njt = n // 128            # key row tiles
    npair = njt // 2          # key-tile pairs per (isup, head)
    is_chunk = min(1024, n)   # q rows per attention sweep
    nis = n // is_chunk
    n_half = is_chunk // 512

    import contextlib
    with contextlib.ExitStack() as ctx:
        singles = ctx.enter_context(tc.tile_pool(name="singles", bufs=1))
        qk_stream = ctx.enter_context(tc.tile_pool(name="qk_stream", bufs=24))
        vt_pool = ctx.enter_context(tc.tile_pool(name="vt_pool", bufs=3))
        pt_pool = ctx.enter_context(tc.tile_pool(name="pt_pool", bufs=7))
        ostage = ctx.enter_context(tc.tile_pool(name="ostage", bufs=6))
        u_pool = ctx.enter_context(tc.tile_pool(name="u_pool", bufs=2))
        u1n_pool = ctx.enter_context(tc.tile_pool(name="u1n_pool", bufs=2))
        ustack_pool = ctx.enter_context(tc.tile_pool(name="ustack", bufs=2))
        nrm_pool = ctx.enter_context(tc.tile_pool(name="nrm_pool", bufs=2))
        ps_scores = ctx.enter_context(
            tc.tile_pool(name="ps_scores", bufs=2, space="PSUM"))
        ps_acc = ctx.enter_context(
            tc.tile_pool(name="ps_acc", bufs=1, space="PSUM"))
        ps_small = ctx.enter_context(
            tc.tile_pool(name="ps_small", bufs=2, space="PSUM"))

        # ---- weights to SBUF (identity first: gates the PE warm-up) ----
        ident_sb = singles.tile([128, 128], F16, tag="ident")
        nc.sync.dma_start(out=ident_sb, in_=ident)
        wq_sb = singles.tile([128, nkt, 128], F16, tag="wq")
        wk_sb = singles.tile([128, nkt, 128], F16, tag="wk")
        wv_sb = singles.tile([128, nkt, 128], F16, tag="wv")
        for w_sb, w in ((wq_sb, wq), (wk_sb, wk), (wv_sb, wv)):
            nc.sync.dma_start(out=w_sb,
                              in_=w.rearrange("(kt p) c -> p kt c", p=128))
        wo_sb = singles.tile([128, DOUT], F16, tag="wo")
        nc.sync.dma_start(out=wo_sb, in_=wo)
        # ones row at partition 64 (for the K=1 broadcast matmul)
        ones_sb = singles.tile([65, 64], F16, tag="ones")
        nc.vector.memset(ones_sb[64:65, :], 1.0)

        # ---- V-tile slots: allocate early, memset ones columns off the
        # critical path (DVE is idle during warm-up) ----
        v_slots = []
        for jt in range(njt):
            v_t = singles.tile([128, 130], F16, tag=f"v{jt}", name=f"v{jt}")
            nc.vector.memset(v_t[:, 64:65], 1.0)
            nc.vector.memset(v_t[:, 129:130], 1.0)
            v_slots.append(v_t)

        # ---- PE warm-up: unthrottle HAM before the projection burst ----
        junk = ps_small.tile([128, 128], F32, tag="w", name="junk")
        for _ in range(36):
            nc.tensor.matmul(junk, lhsT=ident_sb, rhs=ident_sb,
                             start=True, stop=True)

        # ---- projection micro-unit generators ----
        qt_pairs = [None] * nis     # [128, is_chunk] h0 rows 0-63, h1 64-127
        qt_dups = [None] * nis      # partition-swapped duplicate
        kt_tiles = [None] * nch     # [128, 512]
        kt_dups = [None] * nch
        v_tiles = [None] * njt      # [128, 130] row-major V0|1|V1|1

        def qk_chunk(src, w_sb, store, dups, tagp, i, pair_of=None):
            """Project one 512-col chunk; yields micro-units."""
            box = []
            for kt in range(nkt):
                def unit(kt=kt):
                    if kt == 0:
                        box.append(ps_small.tile([128, 512], F32, tag="w",
                                                 name=f"ps_{tagp}{i}"))
                    st = qk_stream.tile([128, 512], F16, tag="qkst",
                                        name=f"st_{tagp}{i}_{kt}")
                    dma_eng = nc.sync if kt % 2 == 0 else nc.gpsimd
                    dma_eng.dma_start(
                        out=st,
                        in_=src[kt * 128:(kt + 1) * 128, i * 512:(i + 1) * 512])
                    nc.tensor.matmul(box[0], lhsT=w_sb[:, kt, :], rhs=st,
                                     start=(kt == 0), stop=(kt == nkt - 1))
                yield unit

            def fin():
                if pair_of is not None:
                    tiles, pi, half = pair_of
                    if tiles[pi] is None:
                        tiles[pi] = singles.tile([128, is_chunk], F16,
                                                 tag=f"{tagp}p{pi}",
                                                 name=f"{tagp}p{pi}")
                    nc.vector.tensor_copy(
                        tiles[pi][:, half * 512:(half + 1) * 512], box[0])
                    if half == n_half - 1:
                        # partition-swapped duplicate for quadrant pairing
                        qt_dups[pi] = singles.tile([128, is_chunk], F16,
                                                   tag=f"qtd{pi}",
                                                   name=f"qtd{pi}")
                        nc.sync.dma_start(out=qt_dups[pi][64:128, :],
                                          in_=tiles[pi][0:64, :])
                        nc.gpsimd.dma_start(out=qt_dups[pi][0:64, :],
                                            in_=tiles[pi][64:128, :])
                    return
                pool = singles if store is not None else vt_pool
                dst = pool.tile(
                    [128, 512], F16,
                    tag=(f"{tagp}{i}" if store is not None else "vtc"),
                    name=f"{tagp}{i}")
                nc.vector.tensor_copy(dst, box[0])
                if store is not None:
                    store[i] = dst
                    dups[i] = singles.tile([128, 512], F16, tag=f"ktd{i}",
                                           name=f"ktd{i}")
                    nc.sync.dma_start(out=dups[i][64:128, :],
                                      in_=dst[0:64, :])
                    nc.gpsimd.dma_start(out=dups[i][0:64, :],
                                        in_=dst[64:128, :])
                else:
                    box.append(dst)
            yield fin
            if store is None and pair_of is None:
                # V: transpose each 128-row block to row-major V0|1|V1|1
                for r in range(4):
                    def tunit(r=r):
                        jt = 4 * i + r
                        vtc = box[1]
                        tp = ps_small.tile([128, 128], F16, tag="w",
                                           name=f"tp{jt}")
                        nc.tensor.transpose(tp, vtc[:, r * 128:(r + 1) * 128],
                                            ident_sb)
                        v_t = v_slots[jt]
                        nc.vector.tensor_copy(v_t[:, 0:64], tp[:, 0:64])
                        nc.vector.tensor_copy(v_t[:, 65:129], tp[:, 64:128])
                        v_tiles[jt] = v_t
                    yield tunit

        # up-front: what the first pair-steps need; the rest drips in.
        up_q = min(2, nch)
        for u_ in qk_chunk(kT, wk_sb, kt_tiles, kt_dups, "kt", 0):
            u_()
        for u_ in qk_chunk(vT, wv_sb, None, None, "vt", 0):
            u_()

        def q_chunk(i):
            return qk_chunk(qT, wq_sb, None, None, "qt", i,
                            pair_of=(qt_pairs, i // n_half, i % n_half))
        for i in range(up_q):
            for u_ in q_chunk(i):
                u_()
        drip = []
        for i in range(1, nch):
            drip.extend(qk_chunk(kT, wk_sb, kt_tiles, kt_dups, "kt", i))
            drip.extend(qk_chunk(vT, wv_sb, None, None, "vt", i))
        for i in range(up_q, nch):
            drip.extend(q_chunk(i))
        drip.reverse()  # pop() from the end

        # ---- score matmul operand selection -----------------------------
        def sc_ops(h, jt, isup):
            """(lhsT, rhs) putting this (h, jt) on quadrant row jt%2*64."""
            ks = slice((jt % 4) * 128, (jt % 4) * 128 + 128)
            kt = kt_tiles[jt // 4]
            ktd = kt_dups[jt // 4]
            qt = qt_pairs[isup]
            qtd = qt_dups[isup]
            lo, hi = slice(0, 64), slice(64, 128)
            if h == 0:
                if jt % 2 == 0:
                    return kt[lo, ks], qt[lo, :]
                return ktd[hi, ks], qtd[hi, :]
            if jt % 2 == 0:
                return ktd[lo, ks], qtd[lo, :]
            return kt[hi, ks], qt[hi, :]

        # ---- deferred epilogue units ------------------------------------
        def norm_unit(uraw, half, dst, tag):
            """normalize rows 0-63 of uraw into dst (same partitions)."""
            def unit():
                sl = slice(half * 512, (half + 1) * 512)
                bc_ps = ps_small.tile([64, 512], F32, tag="w",
                                      name=f"bc{tag}_{half}")
                nc.tensor.matmul(bc_ps, lhsT=ones_sb[64:65, :],
                                 rhs=uraw[64:65, sl], start=True, stop=True)
                rbc = nrm_pool.tile([64, 512], F32, tag="rbc",
                                    name=f"rbc{tag}_{half}")
                nc.vector.reciprocal_approx_fast(out=rbc, in_=bc_ps)
                nc.vector.tensor_mul(dst[:, sl], uraw[0:64, sl], rbc)
            return unit

        def out_unit(ustack, isup, it):
            def unit():
                row0 = isup * is_chunk + it * 128
                for wc in range(DOUT // 512):
                    po = ps_small.tile([128, 512], F32, tag="w",
                                       name=f"po{isup}_{it}_{wc}")
                    nc.tensor.matmul(po,
                                     lhsT=ustack[:, it * 128:(it + 1) * 128],
                                     rhs=wo_sb[:, wc * 512:(wc + 1) * 512],
                                     start=True, stop=True)
                    ot = ostage.tile([128, 512], F32, tag="ot",
                                     name=f"ot{isup}_{it}_{wc}")
                    nc.vector.tensor_copy(ot, po)
                    dma_eng = nc.gpsimd if (it + wc) % 2 == 0 else nc.sync
                    dma_eng.dma_start(
                        out=out[row0:row0 + 128, wc * 512:(wc + 1) * 512],
                        in_=ot)
            return unit

        # ---- software-pipelined attention over pair-steps ----------------
        steps = [(isup, h, p)
                 for isup in range(nis) for h in range(2)
                 for p in range(npair)]
        pending_v = []   # up to 2 entries: (acc, h, (jt0, jt1), pts)
        drains = []      # (due_step, fn)
        epi = []         # (due_step, fn)
        exp_count = [0]
        ustacks = [None] * nis
        accs = {}

        def emit_exp(pt_dst, sc_src):
            c = exp_count[0]
            exp_count[0] += 1
            if c % DVE_EXP_MOD == DVE_EXP_MOD - 1:
                nc.vector.tensor_scalar(out=pt_dst.bitcast(I16), in0=sc_src,
                                        scalar1=SCH_A, scalar2=SCH_B,
                                        op0=alu.AluOpType.mult,
                                        op1=alu.AluOpType.add)
            else:
                nc.scalar.activation(pt_dst, sc_src, EXP, scale=0.125)

        def flush_v_half(ji):
            acc_p, h_p, jts_p, pts_p = pending_v[0]
            vlo = 0 if h_p == 0 else 65
            jt = jts_p[ji]
            for half in range(n_half):
                nc.tensor.matmul(
                    acc_p[:, half * 512:(half + 1) * 512],
                    lhsT=v_tiles[jt][:, vlo:vlo + 65],
                    rhs=pts_p[half][:, ji * 512:(ji + 1) * 512],
                    start=(jt == 0), stop=(jt == njt - 1))

        def flush_v():
            flush_v_half(0)
            flush_v_half(1)
            pending_v.pop(0)

        def need(pred):
            while not pred():
                drip.pop()()

        for t, (isup, h, p) in enumerate(steps):
            if p == 0:
                if h == 0:
                    ustacks[isup] = ustack_pool.tile(
                        [128, is_chunk], F16, tag="us", name=f"us{isup}")
                accs[(isup, h)] = ps_acc.tile([65, is_chunk], F32, tag="acc",
                                              name=f"acc{isup}_{h}")
            jts = (2 * p, 2 * p + 1)
            ch = jts[1] // 4
            need(lambda: kt_tiles[ch] is not None
                 and kt_dups[ch] is not None
                 and qt_pairs[isup] is not None
                 and qt_dups[isup] is not None)
            if len(pending_v) == 2:
                vlast = pending_v[0][2][1]
                need(lambda: v_tiles[vlast] is not None)
            # scores: X Y Y X quadrant order — adjacent pairs overlap on
            # the PE and the middle Y pair shares its K weight load.
            # sc tile for query-half H holds [jt0 cols | jt1 cols].
            # scores: X Y Y X quadrant order — adjacent pairs overlap on
            # the PE and the middle Y pair shares its K weight load.
            # sc tile for query-half H holds [jt0 cols | jt1 cols].
            scs = []
            for half in range(n_half):
                scs.append(ps_scores.tile(
                    [128, 1024], F32, tag="s",
                    name=f"sc{isup}_{h}_{p}_{half}"))
            # V chains of pair p-2 FIRST: they chain tightly after the
            # previous step's scores (covering their pipeline tails) and
            # give the exp engines a full step of slack before the score
            # tiles are reused.
            if len(pending_v) == 2:
                flush_v()
            # projection drip + deferred epilogue fill more PE time here
            for _ in range(8 if t < 16 else (5 if t < 28 else 2)):
                if drip:
                    drip.pop()()
            while drains and drains[0][0] <= t:
                drains.pop(0)[1]()
            if epi and epi[0][0] <= t:
                epi.pop(0)[1]()
            for half, ji in ((0, 0), (0, 1), (1, 1), (1, 0)):
                jt = jts[ji]
                lhsT, rhs = sc_ops(h, jt, isup)
                nc.tensor.matmul(
                    scs[half][:, ji * 512:(ji + 1) * 512], lhsT=lhsT,
                    rhs=rhs[:, half * 512:(half + 1) * 512],
                    start=True, stop=True)
            # exp: one 1024-free unit per query-half
            pts = []
            for half in range(n_half):
                pt = pt_pool.tile([128, 1024], F16, tag="pt",
                                  name=f"pt{isup}_{h}_{p}_{half}")
                emit_exp(pt, scs[half])
                pts.append(pt)
            acc = accs[(isup, h)]
            pending_v.append((acc, h, jts, pts))
            if p == npair - 1:
                # head seam: drain + normalize later (V still pending)
                uraw = u_pool.tile([65, is_chunk], F16, tag=f"u{h}",
                                   name=f"uraw{isup}_{h}")
                ustack = ustacks[isup]
                if h == 1:
                    u1n = u1n_pool.tile([64, is_chunk], F16, tag="u1n",
                                        name=f"u1n{isup}")

                def drain(acc=acc, uraw=uraw):
                    nc.vector.tensor_copy(uraw, acc)
                    for _ in range(4):
                        nc.tensor.matmul(junk, lhsT=ident_sb, rhs=ident_sb,
                                         start=True, stop=True)
                drains.append((t + 2, drain))
                if h == 0:
                    nunits = [(t + 3, norm_unit(uraw, half, ustack[0:64, :],
                                                f"{isup}_0"))
                              for half in range(n_half)]
                else:
                    nunits = [(t + 3, norm_unit(uraw, half, u1n,
                                                f"{isup}_1"))
                              for half in range(n_half)]

                    def shift(u1n=u1n, ustack=ustack):
                        nc.sync.dma_start(out=ustack[64:128, :], in_=u1n)
                    nunits.append((t + 4, shift))
                    nunits.extend(
                        (t + 5 + it % 2, out_unit(ustack, isup, it))
                        for it in range(is_chunk // 128))
                if t + 1 < len(steps):
                    epi.extend(nunits)
                else:
                    tail_epi = nunits

        # ---- tail: last head's V, drains, normalize + out-proj ----------
        while pending_v:
            flush_v()
        while drip:
            drip.pop()()
        for _, fn in drains:
            fn()
        for _ in range(10):
            nc.tensor.matmul(junk, lhsT=ident_sb, rhs=ident_sb,
                             start=True, stop=True)
        for _, fn in epi:
            fn()
        for _, fn in tail_epi:
            fn()


def build(n=N, din=DIN):
    nc = bacc.Bacc("TRN2", target_bir_lowering=False, debug=False,
                   num_devices=NCORES)
    qT = nc.dram_tensor("qT", [din, n], F16, kind="ExternalInput").ap()
    kT = nc.dram_tensor("kT", [din, n], F16, kind="ExternalInput").ap()
    vT = nc.dram_tensor("vT", [din, n], F16, kind="ExternalInput").ap()
    wq = nc.dram_tensor("wq", [din, DPC], F16, kind="ExternalInput").ap()
    wk = nc.dram_tensor("wk", [din, DPC], F16, kind="ExternalInput").ap()
    wv = nc.dram_tensor("wv", [din, DPC], F16, kind="ExternalInput").ap()
    wo = nc.dram_tensor("wo", [DPC, DOUT], F16, kind="ExternalInput").ap()
    ident = nc.dram_tensor("ident", [128, 128], F16, kind="ExternalInput").ap()
    out = nc.dram_tensor("out", [n, DOUT], F32, kind="ExternalOutput").ap()
    with tile.TileContext(nc) as tc:
        emit(tc, [out], [qT, kT, vT, wq, wk, wv, wo, ident], n, din)
    nc.compile()
    return nc


_NC_CACHE = {}


def _get_nc(n=N, din=DIN):
    key = (n, din)
    if key not in _NC_CACHE:
        _NC_CACHE[key] = build(n, din)
    return _NC_CACHE[key]


def make_in_maps(q, k, v, W_q, W_k, W_v, W_o):
    f16 = np.float16
    qT = np.ascontiguousarray(np.asarray(q, dtype=np.float32).T).astype(f16)
    kT = np.ascontiguousarray(np.asarray(k, dtype=np.float32).T).astype(f16)
    vT = np.ascontiguousarray(np.asarray(v, dtype=np.float32).T).astype(f16)
    W_q = np.asarray(W_q, dtype=np.float32)
    W_k = np.asarray(W_k, dtype=np.float32)
    W_v = np.asarray(W_v, dtype=np.float32)
    W_o = np.asarray(W_o, dtype=np.float32)
    ident = np.eye(128, dtype=f16)
    in_maps = []
    for c in range(NCORES):
        sl = slice(DPC * c, DPC * (c + 1))
        in_maps.append({
            "qT": qT, "kT": kT, "vT": vT,
            "wq": np.ascontiguousarray(W_q[:, sl]).astype(f16),
            "wk": np.ascontiguousarray(W_k[:, sl]).astype(f16),
            "wv": np.ascontiguousarray(W_v[:, sl]).astype(f16),
            "wo": np.ascontiguousarray(W_o[sl, :]).astype(f16),
            "ident": ident,
        })
    return in_maps


def run(q, k, v, W_q, W_k, W_v, W_o, trace=False):
    n = q.shape[0]
    nc = _get_nc(n=n, din=q.shape[1])
    in_maps = make_in_maps(q, k, v, W_q, W_k, W_v, W_o)
    res = run_bass_kernel_spmd(nc, in_maps, list(range(NCORES)), trace=trace)
    out = res.results[0]["out"].astype(np.float32)
    for c in range(1, NCORES):
        out += res.results[c]["out"]
    return out, res


def kernel(q, k, v, W_q, W_k, W_v, W_o):
    out, _ = run(q, k, v, W_q, W_k, W_v, W_o)
    return out


# revision 12
# speedup vs baseline: 1.0224x; 1.0224x over previous
"""Multi-head attention (N=4096, D=1024, 16 heads) on 8 trn2 NeuronCores.

Sharding: tensor-parallel over heads. Each core owns 2 heads (128 of the
1024 QKV projection columns / W_o rows), computes its heads' attention
fully on-device, applies its W_o row-slice, and returns a partial
[4096, 1024] output. The host sums the 8 partials (the "all-reduce").

Device kernel per core (fp16 matmuls, fp32 PSUM accumulation):
  1. QT/KT/VT = (W^T x^T) chunks [128 = 2*64 head dims, 512 cols]; V is
     PE-transposed to row-major [row 128, V0 | 1 | V1 | 1]. K and Q get
     partition-swapped duplicates (via SBUF->SBUF DMA) so each head's
     K/Q slice exists on BOTH partition halves.
  2. Attention sweeps 1024 queries (isup) x head x key-tile-PAIR. The
     two score matmul groups of a pair run on opposite PE quadrants
     (rows 0-63 vs 64-127, via the duplicate tiles) and the hardware
     executes them CONCURRENTLY (~2x on scores). exp() is split between
     ScalarE (activation table) and DVE (Schraudolph int16 bit-trick,
     zero-mean calibrated) to keep both under the PE roofline.
  3. u accumulates in PSUM [65, 1024] (V|1 rows); at head seams u is
     drained, normalized (broadcast-matmul + reciprocal + multiply) into
     a head-stacked [128, 1024] tile (h1 partition-shifted by DMA), and
     projected with ONE contraction-128 matmul per output tile.
"""

import numpy as np
import ml_dtypes  # noqa: F401

import concourse.bass as bass  # noqa: F401
import concourse.tile as tile
from concourse import bacc, mybir
from concourse.bass_utils import run_bass_kernel_spmd
import concourse.alu_op_type as alu

F16 = mybir.dt.float16
F32 = mybir.dt.float32
I16 = mybir.dt.int16
EXP = mybir.ActivationFunctionType.Exp

N = 4096
DIN = 1024
DOUT = 1024
NCORES = 8
DPC = 128  # dims per core = 2 heads * 64
HD = 64

# Schraudolph exp on DVE: int16(round(s*SCH_A + SCH_B)) bits viewed as
# fp16 == exp(s/8) * (1 + eps), eps zero-mean, rms 1.8%, |eps| < 4%.
SCH_A = 0.125 / float(np.log(2.0)) * 1024.0
SCH_B = 15360.0 - 59.0
DVE_EXP_MOD = 6  # every 6th exp unit goes to DVE (alpha = 1/6)


def emit(tc, outs, ins, n, din):
    nc = tc.nc
    qT, kT, vT, wq, wk, wv, wo, ident = ins
    out = outs[0]

    nkt = din // 128          # contraction tiles for projections
    nch = n // 512            # 512-wide column chunks of QT/KT/VT
    njt = n // 128            # key row tiles
    npair = njt // 2          # key-tile pairs per (isup, head)
    is_chunk = min(1024, n)   # q rows per attention sweep
    nis = n // is_chunk
    n_half = is_chunk // 512

    import contextlib
    with contextlib.ExitStack() as ctx:
        singles = ctx.enter_context(tc.tile_pool(name="singles", bufs=1))
        qk_stream = ctx.enter_context(tc.tile_pool(name="qk_stream", bufs=24))
        vt_pool = ctx.enter_context(tc.tile_pool(name="vt_pool", bufs=3))
        pt_pool = ctx.enter_context(tc.tile_pool(name="pt_pool", bufs=7))
        ostage = ctx.enter_context(tc.tile_pool(name="ostage", bufs=6))
        u_pool = ctx.enter_context(tc.tile_pool(name="u_pool", bufs=2))
        u1n_pool = ctx.enter_context(tc.tile_pool(name="u1n_pool", bufs=2))
        ustack_pool = ctx.enter_context(tc.tile_pool(name="ustack", bufs=2))
        nrm_pool = ctx.enter_context(tc.tile_pool(name="nrm_pool", bufs=2))
        ps_scores = ctx.enter_context(
            tc.tile_pool(name="ps_scores", bufs=2, space="PSUM"))
        ps_acc = ctx.enter_context(
            tc.tile_pool(name="ps_acc", bufs=1, space="PSUM"))
        ps_small = ctx.enter_context(
            tc.tile_pool(name="ps_small", bufs=2, space="PSUM"))

        # ---- weights to SBUF (identity first: gates the PE warm-up) ----
        ident_sb = singles.tile([128, 128], F16, tag="ident")
        nc.sync.dma_start(out=ident_sb, in_=ident)
        wq_sb = singles.tile([128, nkt, 128], F16, tag="wq")
        wk_sb = singles.tile([128, nkt, 128], F16, tag="wk")
        wv_sb = singles.tile([128, nkt, 128], F16, tag="wv")
        for w_sb, w in ((wq_sb, wq), (wk_sb, wk), (wv_sb, wv)):
            nc.sync.dma_start(out=w_sb,
                              in_=w.rearrange("(kt p) c -> p kt c", p=128))
        wo_sb = singles.tile([128, DOUT], F16, tag="wo")
        nc.sync.dma_start(out=wo_sb, in_=wo)
        # ones row at partition 64 (for the K=1 broadcast matmul)
        ones_sb = singles.tile([65, 64], F16, tag="ones")
        nc.vector.memset(ones_sb[64:65, :], 1.0)

        # ---- V-tile slots: allocate early, memset ones columns off the
        # critical path (DVE is idle during warm-up) ----
        v_slots = []
        for jt in range(njt):
            v_t = singles.tile([128, 130], F16, tag=f"v{jt}", name=f"v{jt}")
            nc.vector.memset(v_t[:, 64:65], 1.0)
            nc.vector.memset(v_t[:, 129:130], 1.0)
            v_slots.append(v_t)

        # ---- PE warm-up: unthrottle HAM before the projection burst ----
        junk = ps_small.tile([128, 128], F32, tag="w", name="junk")
        for _ in range(36):
            nc.tensor.matmul(junk, lhsT=ident_sb, rhs=ident_sb,
                             start=True, stop=True)

        # ---- projection micro-unit generators ----
        qt_pairs = [None] * nis     # [128, is_chunk] h0 rows 0-63, h1 64-127
        qt_dups = [None] * nis      # partition-swapped duplicate
        kt_tiles = [None] * nch     # [128, 512]
        kt_dups = [None] * nch
        v_tiles = [None] * njt      # [128, 130] row-major V0|1|V1|1

        def qk_chunk(src, w_sb, store, dups, tagp, i, pair_of=None):
            """Project one 512-col chunk; yields micro-units."""
            box = []
            for kt in range(nkt):
                def unit(kt=kt):
                    if kt == 0:
                        box.append(ps_small.tile([128, 512], F32, tag="w",
                                                 name=f"ps_{tagp}{i}"))
                    st = qk_stream.tile([128, 512], F16, tag="qkst",
                                        name=f"st_{tagp}{i}_{kt}")
                    dma_eng = nc.sync if kt % 2 == 0 else nc.gpsimd
                    dma_eng.dma_start(
                        out=st,
                        in_=src[kt * 128:(kt + 1) * 128, i * 512:(i + 1) * 512])
                    nc.tensor.matmul(box[0], lhsT=w_sb[:, kt, :], rhs=st,
                                     start=(kt == 0), stop=(kt == nkt - 1))
                yield unit

            def fin():
                if pair_of is not None:
                    tiles, pi, half = pair_of
                    if tiles[pi] is None:
                        tiles[pi] = singles.tile([128, is_chunk], F16,
                                                 tag=f"{tagp}p{pi}",
                                                 name=f"{tagp}p{pi}")
                    nc.vector.tensor_copy(
                        tiles[pi][:, half * 512:(half + 1) * 512], box[0])
                    if half == n_half - 1:
                        # partition-swapped duplicate for quadrant pairing
                        qt_dups[pi] = singles.tile([128, is_chunk], F16,
                                                   tag=f"qtd{pi}",
                                                   name=f"qtd{pi}")
                        nc.sync.dma_start(out=qt_dups[pi][64:128, :],
                                          in_=tiles[pi][0:64, :])
                        nc.gpsimd.dma_start(out=qt_dups[pi][0:64, :],
                                            in_=tiles[pi][64:128, :])
                    return
                pool = singles if store is not None else vt_pool
                dst = pool.tile(
                    [128, 512], F16,
                    tag=(f"{tagp}{i}" if store is not None else "vtc"),
                    name=f"{tagp}{i}")
                nc.vector.tensor_copy(dst, box[0])
                if store is not None:
                    store[i] = dst
                    dups[i] = singles.tile([128, 512], F16, tag=f"ktd{i}",
                                           name=f"ktd{i}")
                    nc.sync.dma_start(out=dups[i][64:128, :],
                                      in_=dst[0:64, :])
                    nc.gpsimd.dma_start(out=dups[i][0:64, :],
                                        in_=dst[64:128, :])
                else:
                    box.append(dst)
            yield fin
            if store is None and pair_of is None:
                # V: transpose each 128-row block to row-major V0|1|V1|1
                for r in range(4):
                    def tunit(r=r):
                        jt = 4 * i + r
                        vtc = box[1]
                        tp = ps_small.tile([128, 128], F16, tag="w",
                                           name=f"tp{jt}")
                        nc.tensor.transpose(tp, vtc[:, r * 128:(r + 1) * 128],
                                            ident_sb)
                        v_t = v_slots[jt]
                        nc.vector.tensor_copy(v_t[:, 0:64], tp[:, 0:64])
                        nc.vector.tensor_copy(v_t[:, 65:129], tp[:, 64:128])
                        v_tiles[jt] = v_t
                    yield tunit

        # up-front: what the first pair-steps need; the rest drips in.
        up_q = min(2, nch)
        for u_ in qk_chunk(kT, wk_sb, kt_tiles, kt_dups, "kt", 0):
            u_()
        for u_ in qk_chunk(vT, wv_sb, None, None, "vt", 0):
            u_()

        def q_chunk(i):
            return qk_chunk(qT, wq_sb, None, None, "qt", i,
                            pair_of=(qt_pairs, i // n_half, i % n_half))
        for i in range(up_q):
            for u_ in q_chunk(i):
                u_()
        drip = []
        for i in range(1, nch):
            drip.extend(qk_chunk(kT, wk_sb, kt_tiles, kt_dups, "kt", i))
            drip.extend(qk_chunk(vT, wv_sb, None, None, "vt", i))
        for i in range(up_q, nch):
            drip.extend(q_chunk(i))
        drip.reverse()  # pop() from the end

        # ---- score matmul operand selection -----------------------------
        def sc_ops(h, jt, isup):
            """(lhsT, rhs) putting this (h, jt) on quadrant row jt%2*64."""
            ks = slice((jt % 4) * 128, (jt % 4) * 128 + 128)
            kt = kt_tiles[jt // 4]
            ktd = kt_dups[jt // 4]
            qt = qt_pairs[isup]
            qtd = qt_dups[isup]
            lo, hi = slice(0, 64), slice(64, 128)
            if h == 0:
                if jt % 2 == 0:
                    return kt[lo, ks], qt[lo, :]
                return ktd[hi, ks], qtd[hi, :]
            if jt % 2 == 0:
                return ktd[lo, ks], qtd[lo, :]
            return kt[hi, ks], qt[hi, :]

        # ---- deferred epilogue units ------------------------------------
        def norm_unit(uraw, half, dst, tag):
            """normalize rows 0-63 of uraw into dst (same partitions)."""
            def unit():
                sl = slice(half * 512, (half + 1) * 512)
                bc_ps = ps_small.tile([64, 512], F32, tag="w",
                                      name=f"bc{tag}_{half}")
                nc.tensor.matmul(bc_ps, lhsT=ones_sb[64:65, :],
                                 rhs=uraw[64:65, sl], start=True, stop=True)
                rbc = nrm_pool.tile([64, 512], F32, tag="rbc",
                                    name=f"rbc{tag}_{half}")
                nc.vector.reciprocal_approx_fast(out=rbc, in_=bc_ps)
                nc.vector.tensor_mul(dst[:, sl], uraw[0:64, sl], rbc)
            return unit

        def out_unit(ustack, isup, it):
            def unit():
                row0 = isup * is_chunk + it * 128
                for wc in range(DOUT // 512):
                    po = ps_small.tile([128, 512], F32, tag="w",
                                       name=f"po{isup}_{it}_{wc}")
                    nc.tensor.matmul(po,
                                     lhsT=ustack[:, it * 128:(it + 1) * 128],
                                     rhs=wo_sb[:, wc * 512:(wc + 1) * 512],
                                     start=True, stop=True)
                    ot = ostage.tile([128, 512], F32, tag="ot",
                                     name=f"ot{isup}_{it}_{wc}")
                    nc.vector.tensor_copy(ot, po)
                    dma_eng = nc.gpsimd if (it + wc) % 2 == 0 else nc.sync
                    dma_eng.dma_start(
                        out=out[row0:row0 + 128, wc * 512:(wc + 1) * 512],
                        in_=ot)
            return unit

        # ---- software-pipelined attention over pair-steps ----------------
        steps = [(isup, h, p)
                 for isup in range(nis) for h in range(2)
                 for p in range(npair)]
        pending_v = []   # up to 2 entries: (acc, h, (jt0, jt1), pts)
        drains = []      # (due_step, fn)
        epi = []         # (due_step, fn)
        exp_count = [0]
        ustacks = [None] * nis
        accs = {}

        def emit_exp(pt_dst, sc_src):
            c = exp_count[0]
            exp_count[0] += 1
            if c % DVE_EXP_MOD == DVE_EXP_MOD - 1:
                nc.vector.tensor_scalar(out=pt_dst.bitcast(I16), in0=sc_src,
                                        scalar1=SCH_A, scalar2=SCH_B,
                                        op0=alu.AluOpType.mult,
                                        op1=alu.AluOpType.add)
            else:
                nc.scalar.activation(pt_dst, sc_src, EXP, scale=0.125)

        def flush_v():
            acc_p, h_p, jts_p, pts_p = pending_v.pop(0)
            vlo = 0 if h_p == 0 else 65
            for ji, jt in enumerate(jts_p):
                for half in range(n_half):
                    nc.tensor.matmul(
                        acc_p[:, half * 512:(half + 1) * 512],
                        lhsT=v_tiles[jt][:, vlo:vlo + 65],
                        rhs=pts_p[half][:, ji * 512:(ji + 1) * 512],
                        start=(jt == 0), stop=(jt == njt - 1))

        def need(pred):
            while not pred():
                drip.pop()()

        for t, (isup, h, p) in enumerate(steps):
            if p == 0:
                if h == 0:
                    ustacks[isup] = ustack_pool.tile(
                        [128, is_chunk], F16, tag="us", name=f"us{isup}")
                accs[(isup, h)] = ps_acc.tile([65, is_chunk], F32, tag="acc",
                                              name=f"acc{isup}_{h}")
            jts = (2 * p, 2 * p + 1)
            ch = jts[1] // 4
            need(lambda: kt_tiles[ch] is not None
                 and kt_dups[ch] is not None
                 and qt_pairs[isup] is not None
                 and qt_dups[isup] is not None)
            if len(pending_v) == 2:
                vlast = pending_v[0][2][1]
                need(lambda: v_tiles[vlast] is not None)
            # scores: X Y Y X quadrant order — adjacent pairs overlap on
            # the PE and the middle Y pair shares its K weight load.
            # sc tile for query-half H holds [jt0 cols | jt1 cols].
            # scores: X Y Y X quadrant order — adjacent pairs overlap on
            # the PE and the middle Y pair shares its K weight load.
            # sc tile for query-half H holds [jt0 cols | jt1 cols].
            scs = []
            for half in range(n_half):
                scs.append(ps_scores.tile(
                    [128, 1024], F32, tag="s",
                    name=f"sc{isup}_{h}_{p}_{half}"))
            for half, ji in ((0, 0), (0, 1), (1, 1), (1, 0)):
                jt = jts[ji]
                lhsT, rhs = sc_ops(h, jt, isup)
                nc.tensor.matmul(
                    scs[half][:, ji * 512:(ji + 1) * 512], lhsT=lhsT,
                    rhs=rhs[:, half * 512:(half + 1) * 512],
                    start=True, stop=True)
            # exp: one 1024-free unit per query-half
            pts = []
            for half in range(n_half):
                pt = pt_pool.tile([128, 1024], F16, tag="pt",
                                  name=f"pt{isup}_{h}_{p}_{half}")
                emit_exp(pt, scs[half])
                pts.append(pt)
            # V matmuls of pair-step t-2
            if len(pending_v) == 2:
                flush_v()
            acc = accs[(isup, h)]
            pending_v.append((acc, h, jts, pts))
            # projection drip
            for _ in range(8 if t < 16 else (5 if t < 28 else 2)):
                if drip:
                    drip.pop()()
            # deferred work whose inputs are long ready
            while drains and drains[0][0] <= t:
                drains.pop(0)[1]()
            if epi and epi[0][0] <= t:
                epi.pop(0)[1]()
            if p == npair - 1:
                # head seam: drain + normalize later (V still pending)
                uraw = u_pool.tile([65, is_chunk], F16, tag=f"u{h}",
                                   name=f"uraw{isup}_{h}")
                ustack = ustacks[isup]
                if h == 1:
                    u1n = u1n_pool.tile([64, is_chunk], F16, tag="u1n",
                                        name=f"u1n{isup}")

                def drain(acc=acc, uraw=uraw):
                    nc.vector.tensor_copy(uraw, acc)
                    for _ in range(4):
                        nc.tensor.matmul(junk, lhsT=ident_sb, rhs=ident_sb,
                                         start=True, stop=True)
                drains.append((t + 2, drain))
                if h == 0:
                    nunits = [(t + 3, norm_unit(uraw, half, ustack[0:64, :],
                                                f"{isup}_0"))
                              for half in range(n_half)]
                else:
                    nunits = [(t + 3, norm_unit(uraw, half, u1n,
                                                f"{isup}_1"))
                              for half in range(n_half)]

                    def shift(u1n=u1n, ustack=ustack):
                        nc.sync.dma_start(out=ustack[64:128, :], in_=u1n)
                    nunits.append((t + 4, shift))
                    nunits.extend(
                        (t + 5 + it % 2, out_unit(ustack, isup, it))
                        for it in range(is_chunk // 128))
                if t + 1 < len(steps):
                    epi.extend(nunits)
                else:
                    tail_epi = nunits

        # ---- tail: last head's V, drains, normalize + out-proj ----------
        while pending_v:
            flush_v()
        while drip:
            drip.pop()()
        for _, fn in drains:
            fn()
        for _ in range(10):
            nc.tensor.matmul(junk, lhsT=ident_sb, rhs=ident_sb,
                             start=True, stop=True)
        for _, fn in epi:
            fn()
        for _, fn in tail_epi:
            fn()


def build(n=N, din=DIN):
    nc = bacc.Bacc("TRN2", target_bir_lowering=False, debug=False,
                   num_devices=NCORES)
    qT = nc.dram_tensor("qT", [din, n], F16, kind="ExternalInput").ap()
    kT = nc.dram_tensor("kT", [din, n], F16, kind="ExternalInput").ap()
    vT = nc.dram_tensor("vT", [din, n], F16, kind="ExternalInput").ap()
    wq = nc.dram_tensor("wq", [din, DPC], F16, kind="ExternalInput").ap()
    wk = nc.dram_tensor("wk", [din, DPC], F16, kind="ExternalInput").ap()
    wv = nc.dram_tensor("wv", [din, DPC], F16, kind="ExternalInput").ap()
    wo = nc.dram_tensor("wo", [DPC, DOUT], F16, kind="ExternalInput").ap()
    ident = nc.dram_tensor("ident", [128, 128], F16, kind="ExternalInput").ap()
    out = nc.dram_tensor("out", [n, DOUT], F32, kind="ExternalOutput").ap()
    with tile.TileContext(nc) as tc:
        emit(tc, [out], [qT, kT, vT, wq, wk, wv, wo, ident], n, din)
    nc.compile()
    return nc


_NC_CACHE = {}


def _get_nc(n=N, din=DIN):
    key = (n, din)
    if key not in _NC_CACHE:
        _NC_CACHE[key] = build(n, din)
    return _NC_CACHE[key]


def make_in_maps(q, k, v, W_q, W_k, W_v, W_o):
    f16 = np.float16
    qT = np.ascontiguousarray(np.asarray(q, dtype=np.float32).T).astype(f16)
    kT = np.ascontiguousarray(np.asarray(k, dtype=np.float32).T).astype(f16)
    vT = np.ascontiguousarray(np.asarray(v, dtype=np.float32).T).astype(f16)
    W_q = np.asarray(W_q, dtype=np.float32)
    W_k = np.asarray(W_k, dtype=np.float32)
    W_v = np.asarray(W_v, dtype=np.float32)
    W_o = np.asarray(W_o, dtype=np.float32)
    ident = np.eye(128, dtype=f16)
    in_maps = []
    for c in range(NCORES):
        sl = slice(DPC * c, DPC * (c + 1))
        in_maps.append({
            "qT": qT, "kT": kT, "vT": vT,
            "wq": np.ascontiguousarray(W_q[:, sl]).astype(f16),
            "wk": np.ascontiguousarray(W_k[:, sl]).astype(f16),
            "wv": np.ascontiguousarray(W_v[:, sl]).astype(f16),
            "wo": np.ascontiguousarray(W_o[sl, :]).astype(f16),
            "ident": ident,
        })
    return in_maps


def run(q, k, v, W_q, W_k, W_v, W_o, trace=False):
    n = q.shape[0]
    nc = _get_nc(n=n, din=q.shape[1])
    in_maps = make_in_maps(q, k, v, W_q, W_k, W_v, W_o)
    res = run_bass_kernel_spmd(nc, in_maps, list(range(NCORES)), trace=trace)
    out = res.results[0]["out"].astype(np.float32)
    for c in range(1, NCORES):
        out += res.results[c]["out"]
    return out, res


def kernel(q, k, v, W_q, W_k, W_v, W_o):
    out, _ = run(q, k, v, W_q, W_k, W_v, W_o)
    return out


# revision 13
# speedup vs baseline: 1.0273x; 1.0047x over previous
"""Multi-head attention (N=4096, D=1024, 16 heads) on 8 trn2 NeuronCores.

Sharding: tensor-parallel over heads. Each core owns 2 heads (128 of the
1024 QKV projection columns / W_o rows), computes its heads' attention
fully on-device, applies its W_o row-slice, and returns a partial
[4096, 1024] output. The host sums the 8 partials (the "all-reduce").

Device kernel per core (fp16 matmuls, fp32 PSUM accumulation):
  1. QT/KT/VT = (W^T x^T) chunks [128 = 2*64 head dims, 512 cols]; V is
     PE-transposed to row-major [row 128, V0 | 1 | V1 | 1]. K and Q get
     partition-swapped duplicates (via SBUF->SBUF DMA) so each head's
     K/Q slice exists on BOTH partition halves.
  2. Attention sweeps 1024 queries (isup) x head x key-tile-PAIR. The
     two score matmul groups of a pair run on opposite PE quadrants
     (rows 0-63 vs 64-127, via the duplicate tiles) and the hardware
     executes them CONCURRENTLY (~2x on scores). exp() is split between
     ScalarE (activation table) and DVE (Schraudolph int16 bit-trick,
     zero-mean calibrated) to keep both under the PE roofline.
  3. u accumulates in PSUM [65, 1024] (V|1 rows); at head seams u is
     drained, normalized (broadcast-matmul + reciprocal + multiply) into
     a head-stacked [128, 1024] tile (h1 partition-shifted by DMA), and
     projected with ONE contraction-128 matmul per output tile.
"""

import numpy as np
import ml_dtypes  # noqa: F401

import concourse.bass as bass  # noqa: F401
import concourse.tile as tile
from concourse import bacc, mybir
from concourse.bass_utils import run_bass_kernel_spmd
import concourse.alu_op_type as alu

F16 = mybir.dt.float16
F32 = mybir.dt.float32
I16 = mybir.dt.int16
EXP = mybir.ActivationFunctionType.Exp

N = 4096
DIN = 1024
DOUT = 1024
NCORES = 8
DPC = 128  # dims per core = 2 heads * 64
HD = 64

# Schraudolph exp on DVE: int16(round(s*SCH_A + SCH_B)) bits viewed as
# fp16 == exp(s/8) * (1 + eps), eps zero-mean, rms 1.8%, |eps| < 4%.
SCH_A = 0.125 / float(np.log(2.0)) * 1024.0
SCH_B = 15360.0 - 59.0
DVE_EXP_MOD = 6  # every 6th exp unit goes to DVE (alpha = 1/6)


def emit(tc, outs, ins, n, din):
    nc = tc.nc
    qT, kT, vT, wq, wk, wv, wo, ident = ins
    out = outs[0]

    nkt = din // 128          # contraction tiles for projections
    nch = n // 512            # 512-wide column chunks of QT/KT/VT
    njt = n // 128            # key row tiles
    npair = njt // 2          # key-tile pairs per (isup, head)
    is_chunk = min(1024, n)   # q rows per attention sweep
    nis = n // is_chunk
    n_half = is_chunk // 512

    import contextlib
    with contextlib.ExitStack() as ctx:
        singles = ctx.enter_context(tc.tile_pool(name="singles", bufs=1))
        qk_stream = ctx.enter_context(tc.tile_pool(name="qk_stream", bufs=24))
        vt_pool = ctx.enter_context(tc.tile_pool(name="vt_pool", bufs=3))
        pt_pool = ctx.enter_context(tc.tile_pool(name="pt_pool", bufs=7))
        ostage = ctx.enter_context(tc.tile_pool(name="ostage", bufs=6))
        u_pool = ctx.enter_context(tc.tile_pool(name="u_pool", bufs=2))
        u1n_pool = ctx.enter_context(tc.tile_pool(name="u1n_pool", bufs=2))
        ustack_pool = ctx.enter_context(tc.tile_pool(name="ustack", bufs=2))
        nrm_pool = ctx.enter_context(tc.tile_pool(name="nrm_pool", bufs=2))
        ps_scores = ctx.enter_context(
            tc.tile_pool(name="ps_scores", bufs=2, space="PSUM"))
        ps_acc = ctx.enter_context(
            tc.tile_pool(name="ps_acc", bufs=1, space="PSUM"))
        ps_small = ctx.enter_context(
            tc.tile_pool(name="ps_small", bufs=2, space="PSUM"))

        # ---- weights to SBUF (identity first: gates the PE warm-up) ----
        ident_sb = singles.tile([128, 128], F16, tag="ident")
        nc.sync.dma_start(out=ident_sb, in_=ident)
        wq_sb = singles.tile([128, nkt, 128], F16, tag="wq")
        wk_sb = singles.tile([128, nkt, 128], F16, tag="wk")
        wv_sb = singles.tile([128, nkt, 128], F16, tag="wv")
        for w_sb, w in ((wq_sb, wq), (wk_sb, wk), (wv_sb, wv)):
            nc.sync.dma_start(out=w_sb,
                              in_=w.rearrange("(kt p) c -> p kt c", p=128))
        wo_sb = singles.tile([128, DOUT], F16, tag="wo")
        nc.sync.dma_start(out=wo_sb, in_=wo)
        # ones row at partition 64 (for the K=1 broadcast matmul)
        ones_sb = singles.tile([65, 64], F16, tag="ones")
        nc.vector.memset(ones_sb[64:65, :], 1.0)

        # ---- V-tile slots: allocate early, memset ones columns off the
        # critical path (DVE is idle during warm-up) ----
        v_slots = []
        for jt in range(njt):
            v_t = singles.tile([128, 130], F16, tag=f"v{jt}", name=f"v{jt}")
            nc.vector.memset(v_t[:, 64:65], 1.0)
            nc.vector.memset(v_t[:, 129:130], 1.0)
            v_slots.append(v_t)

        # ---- PE warm-up: unthrottle HAM before the projection burst.
        # Gate on a memset tile (ready in ~100ns) instead of the ident DMA
        # so the ramp overlaps the DMA-ring spin-up (~11us). ----
        warm_sb = singles.tile([128, 128], F16, tag="warm")
        nc.vector.memset(warm_sb, 0.0)
        junk = ps_small.tile([128, 128], F32, tag="w", name="junk")
        for _ in range(36):
            nc.tensor.matmul(junk, lhsT=warm_sb, rhs=warm_sb,
                             start=True, stop=True)

        # ---- projection micro-unit generators ----
        qt_pairs = [None] * nis     # [128, is_chunk] h0 rows 0-63, h1 64-127
        qt_dups = [None] * nis      # partition-swapped duplicate
        kt_tiles = [None] * nch     # [128, 512]
        kt_dups = [None] * nch
        v_tiles = [None] * njt      # [128, 130] row-major V0|1|V1|1

        def qk_chunk(src, w_sb, store, dups, tagp, i, pair_of=None):
            """Project one 512-col chunk; yields micro-units."""
            box = []
            for kt in range(nkt):
                def unit(kt=kt):
                    if kt == 0:
                        box.append(ps_small.tile([128, 512], F32, tag="w",
                                                 name=f"ps_{tagp}{i}"))
                    st = qk_stream.tile([128, 512], F16, tag="qkst",
                                        name=f"st_{tagp}{i}_{kt}")
                    dma_eng = nc.sync if kt % 2 == 0 else nc.gpsimd
                    dma_eng.dma_start(
                        out=st,
                        in_=src[kt * 128:(kt + 1) * 128, i * 512:(i + 1) * 512])
                    nc.tensor.matmul(box[0], lhsT=w_sb[:, kt, :], rhs=st,
                                     start=(kt == 0), stop=(kt == nkt - 1))
                yield unit

            def fin():
                if pair_of is not None:
                    tiles, pi, half = pair_of
                    if tiles[pi] is None:
                        tiles[pi] = singles.tile([128, is_chunk], F16,
                                                 tag=f"{tagp}p{pi}",
                                                 name=f"{tagp}p{pi}")
                    nc.vector.tensor_copy(
                        tiles[pi][:, half * 512:(half + 1) * 512], box[0])
                    if half == n_half - 1:
                        # partition-swapped duplicate for quadrant pairing
                        qt_dups[pi] = singles.tile([128, is_chunk], F16,
                                                   tag=f"qtd{pi}",
                                                   name=f"qtd{pi}")
                        nc.sync.dma_start(out=qt_dups[pi][64:128, :],
                                          in_=tiles[pi][0:64, :])
                        nc.gpsimd.dma_start(out=qt_dups[pi][0:64, :],
                                            in_=tiles[pi][64:128, :])
                    return
                pool = singles if store is not None else vt_pool
                dst = pool.tile(
                    [128, 512], F16,
                    tag=(f"{tagp}{i}" if store is not None else "vtc"),
                    name=f"{tagp}{i}")
                nc.vector.tensor_copy(dst, box[0])
                if store is not None:
                    store[i] = dst
                    dups[i] = singles.tile([128, 512], F16, tag=f"ktd{i}",
                                           name=f"ktd{i}")
                    nc.sync.dma_start(out=dups[i][64:128, :],
                                      in_=dst[0:64, :])
                    nc.gpsimd.dma_start(out=dups[i][0:64, :],
                                        in_=dst[64:128, :])
                else:
                    box.append(dst)
            yield fin
            if store is None and pair_of is None:
                # V: transpose each 128-row block to row-major V0|1|V1|1
                for r in range(4):
                    def tunit(r=r):
                        jt = 4 * i + r
                        vtc = box[1]
                        tp = ps_small.tile([128, 128], F16, tag="w",
                                           name=f"tp{jt}")
                        nc.tensor.transpose(tp, vtc[:, r * 128:(r + 1) * 128],
                                            ident_sb)
                        v_t = v_slots[jt]
                        nc.vector.tensor_copy(v_t[:, 0:64], tp[:, 0:64])
                        nc.vector.tensor_copy(v_t[:, 65:129], tp[:, 64:128])
                        v_tiles[jt] = v_t
                    yield tunit

        # up-front: what the first pair-steps need; the rest drips in.
        up_q = min(2, nch)
        for u_ in qk_chunk(kT, wk_sb, kt_tiles, kt_dups, "kt", 0):
            u_()
        for u_ in qk_chunk(vT, wv_sb, None, None, "vt", 0):
            u_()

        def q_chunk(i):
            return qk_chunk(qT, wq_sb, None, None, "qt", i,
                            pair_of=(qt_pairs, i // n_half, i % n_half))
        for i in range(up_q):
            for u_ in q_chunk(i):
                u_()
        drip = []
        for i in range(1, nch):
            drip.extend(qk_chunk(kT, wk_sb, kt_tiles, kt_dups, "kt", i))
            drip.extend(qk_chunk(vT, wv_sb, None, None, "vt", i))
        for i in range(up_q, nch):
            drip.extend(q_chunk(i))
        drip.reverse()  # pop() from the end

        # ---- score matmul operand selection -----------------------------
        def sc_ops(h, jt, isup):
            """(lhsT, rhs) putting this (h, jt) on quadrant row jt%2*64."""
            ks = slice((jt % 4) * 128, (jt % 4) * 128 + 128)
            kt = kt_tiles[jt // 4]
            ktd = kt_dups[jt // 4]
            qt = qt_pairs[isup]
            qtd = qt_dups[isup]
            lo, hi = slice(0, 64), slice(64, 128)
            if h == 0:
                if jt % 2 == 0:
                    return kt[lo, ks], qt[lo, :]
                return ktd[hi, ks], qtd[hi, :]
            if jt % 2 == 0:
                return ktd[lo, ks], qtd[lo, :]
            return kt[hi, ks], qt[hi, :]

        # ---- deferred epilogue units ------------------------------------
        def norm_unit(uraw, half, dst, tag):
            """normalize rows 0-63 of uraw into dst (same partitions)."""
            def unit():
                sl = slice(half * 512, (half + 1) * 512)
                bc_ps = ps_small.tile([64, 512], F32, tag="w",
                                      name=f"bc{tag}_{half}")
                nc.tensor.matmul(bc_ps, lhsT=ones_sb[64:65, :],
                                 rhs=uraw[64:65, sl], start=True, stop=True)
                rbc = nrm_pool.tile([64, 512], F32, tag="rbc",
                                    name=f"rbc{tag}_{half}")
                nc.vector.reciprocal_approx_fast(out=rbc, in_=bc_ps)
                nc.vector.tensor_mul(dst[:, sl], uraw[0:64, sl], rbc)
            return unit

        def out_unit(ustack, isup, it):
            def unit():
                row0 = isup * is_chunk + it * 128
                for wc in range(DOUT // 512):
                    po = ps_small.tile([128, 512], F32, tag="w",
                                       name=f"po{isup}_{it}_{wc}")
                    nc.tensor.matmul(po,
                                     lhsT=ustack[:, it * 128:(it + 1) * 128],
                                     rhs=wo_sb[:, wc * 512:(wc + 1) * 512],
                                     start=True, stop=True)
                    ot = ostage.tile([128, 512], F32, tag="ot",
                                     name=f"ot{isup}_{it}_{wc}")
                    nc.vector.tensor_copy(ot, po)
                    dma_eng = nc.gpsimd if (it + wc) % 2 == 0 else nc.sync
                    dma_eng.dma_start(
                        out=out[row0:row0 + 128, wc * 512:(wc + 1) * 512],
                        in_=ot)
            return unit

        # ---- software-pipelined attention over pair-steps ----------------
        steps = [(isup, h, p)
                 for isup in range(nis) for h in range(2)
                 for p in range(npair)]
        pending_v = []   # up to 2 entries: (acc, h, (jt0, jt1), pts)
        drains = []      # (due_step, fn)
        epi = []         # (due_step, fn)
        exp_count = [0]
        ustacks = [None] * nis
        accs = {}

        def emit_exp(pt_dst, sc_src):
            c = exp_count[0]
            exp_count[0] += 1
            if c % DVE_EXP_MOD == DVE_EXP_MOD - 1:
                nc.vector.tensor_scalar(out=pt_dst.bitcast(I16), in0=sc_src,
                                        scalar1=SCH_A, scalar2=SCH_B,
                                        op0=alu.AluOpType.mult,
                                        op1=alu.AluOpType.add)
            else:
                nc.scalar.activation(pt_dst, sc_src, EXP, scale=0.125)

        def flush_v():
            acc_p, h_p, jts_p, pts_p = pending_v.pop(0)
            vlo = 0 if h_p == 0 else 65
            for ji, jt in enumerate(jts_p):
                for half in range(n_half):
                    nc.tensor.matmul(
                        acc_p[:, half * 512:(half + 1) * 512],
                        lhsT=v_tiles[jt][:, vlo:vlo + 65],
                        rhs=pts_p[half][:, ji * 512:(ji + 1) * 512],
                        start=(jt == 0), stop=(jt == njt - 1))

        def need(pred):
            while not pred():
                drip.pop()()

        for t, (isup, h, p) in enumerate(steps):
            if p == 0:
                if h == 0:
                    ustacks[isup] = ustack_pool.tile(
                        [128, is_chunk], F16, tag="us", name=f"us{isup}")
                accs[(isup, h)] = ps_acc.tile([65, is_chunk], F32, tag="acc",
                                              name=f"acc{isup}_{h}")
            jts = (2 * p, 2 * p + 1)
            ch = jts[1] // 4
            need(lambda: kt_tiles[ch] is not None
                 and kt_dups[ch] is not None
                 and qt_pairs[isup] is not None
                 and qt_dups[isup] is not None)
            if len(pending_v) == 2:
                vlast = pending_v[0][2][1]
                need(lambda: v_tiles[vlast] is not None)
            # scores: X Y Y X quadrant order — adjacent pairs overlap on
            # the PE and the middle Y pair shares its K weight load.
            # sc tile for query-half H holds [jt0 cols | jt1 cols].
            # scores: X Y Y X quadrant order — adjacent pairs overlap on
            # the PE and the middle Y pair shares its K weight load.
            # sc tile for query-half H holds [jt0 cols | jt1 cols].
            scs = []
            for half in range(n_half):
                scs.append(ps_scores.tile(
                    [128, 1024], F32, tag="s",
                    name=f"sc{isup}_{h}_{p}_{half}"))
            for half, ji in ((0, 0), (0, 1), (1, 1), (1, 0)):
                jt = jts[ji]
                lhsT, rhs = sc_ops(h, jt, isup)
                nc.tensor.matmul(
                    scs[half][:, ji * 512:(ji + 1) * 512], lhsT=lhsT,
                    rhs=rhs[:, half * 512:(half + 1) * 512],
                    start=True, stop=True)
            # exp: one 1024-free unit per query-half
            pts = []
            for half in range(n_half):
                pt = pt_pool.tile([128, 1024], F16, tag="pt",
                                  name=f"pt{isup}_{h}_{p}_{half}")
                emit_exp(pt, scs[half])
                pts.append(pt)
            # V matmuls of pair-step t-2
            if len(pending_v) == 2:
                flush_v()
            acc = accs[(isup, h)]
            pending_v.append((acc, h, jts, pts))
            # projection drip
            for _ in range(8 if t < 16 else (5 if t < 28 else 2)):
                if drip:
                    drip.pop()()
            # deferred work whose inputs are long ready
            while drains and drains[0][0] <= t:
                drains.pop(0)[1]()
            if epi and epi[0][0] <= t:
                epi.pop(0)[1]()
            if p == npair - 1:
                # head seam: drain + normalize later (V still pending)
                uraw = u_pool.tile([65, is_chunk], F16, tag=f"u{h}",
                                   name=f"uraw{isup}_{h}")
                ustack = ustacks[isup]
                if h == 1:
                    u1n = u1n_pool.tile([64, is_chunk], F16, tag="u1n",
                                        name=f"u1n{isup}")

                def drain(acc=acc, uraw=uraw):
                    nc.vector.tensor_copy(uraw, acc)
                    for _ in range(4):
                        nc.tensor.matmul(junk, lhsT=warm_sb, rhs=warm_sb,
                                         start=True, stop=True)
                drains.append((t + 2, drain))
                if h == 0:
                    nunits = [(t + 3, norm_unit(uraw, half, ustack[0:64, :],
                                                f"{isup}_0"))
                              for half in range(n_half)]
                else:
                    nunits = [(t + 3, norm_unit(uraw, half, u1n,
                                                f"{isup}_1"))
                              for half in range(n_half)]

                    def shift(u1n=u1n, ustack=ustack):
                        nc.sync.dma_start(out=ustack[64:128, :], in_=u1n)
                    nunits.append((t + 4, shift))
                    nunits.extend(
                        (t + 5 + it % 2, out_unit(ustack, isup, it))
                        for it in range(is_chunk // 128))
                if t + 1 < len(steps):
                    epi.extend(nunits)
                else:
                    tail_epi = nunits

        # ---- tail: last head's V, drains, normalize + out-proj ----------
        while pending_v:
            flush_v()
        while drip:
            drip.pop()()
        for _, fn in drains:
            fn()
        for _ in range(10):
            nc.tensor.matmul(junk, lhsT=warm_sb, rhs=warm_sb,
                             start=True, stop=True)
        for _, fn in epi:
            fn()
        for _, fn in tail_epi:
            fn()


def build(n=N, din=DIN):
    nc = bacc.Bacc("TRN2", target_bir_lowering=False, debug=False,
                   num_devices=NCORES)
    qT = nc.dram_tensor("qT", [din, n], F16, kind="ExternalInput").ap()
    kT = nc.dram_tensor("kT", [din, n], F16, kind="ExternalInput").ap()
    vT = nc.dram_tensor("vT", [din, n], F16, kind="ExternalInput").ap()
    wq = nc.dram_tensor("wq", [din, DPC], F16, kind="ExternalInput").ap()
    wk = nc.dram_tensor("wk", [din, DPC], F16, kind="ExternalInput").ap()
    wv = nc.dram_tensor("wv", [din, DPC], F16, kind="ExternalInput").ap()
    wo = nc.dram_tensor("wo", [DPC, DOUT], F16, kind="ExternalInput").ap()
    ident = nc.dram_tensor("ident", [128, 128], F16, kind="ExternalInput").ap()
    out = nc.dram_tensor("out", [n, DOUT], F32, kind="ExternalOutput").ap()
    with tile.TileContext(nc) as tc:
        emit(tc, [out], [qT, kT, vT, wq, wk, wv, wo, ident], n, din)
    nc.compile()
    return nc


_NC_CACHE = {}


def _get_nc(n=N, din=DIN):
    key = (n, din)
    if key not in _NC_CACHE:
        _NC_CACHE[key] = build(n, din)
    return _NC_CACHE[key]


def make_in_maps(q, k, v, W_q, W_k, W_v, W_o):
    f16 = np.float16
    qT = np.ascontiguousarray(np.asarray(q, dtype=np.float32).T).astype(f16)
    kT = np.ascontiguousarray(np.asarray(k, dtype=np.float32).T).astype(f16)
    vT = np.ascontiguousarray(np.asarray(v, dtype=np.float32).T).astype(f16)
    W_q = np.asarray(W_q, dtype=np.float32)
    W_k = np.asarray(W_k, dtype=np.float32)
    W_v = np.asarray(W_v, dtype=np.float32)
    W_o = np.asarray(W_o, dtype=np.float32)
    ident = np.eye(128, dtype=f16)
    in_maps = []
    for c in range(NCORES):
        sl = slice(DPC * c, DPC * (c + 1))
        in_maps.append({
            "qT": qT, "kT": kT, "vT": vT,
            "wq": np.ascontiguousarray(W_q[:, sl]).astype(f16),
            "wk": np.ascontiguousarray(W_k[:, sl]).astype(f16),
            "wv": np.ascontiguousarray(W_v[:, sl]).astype(f16),
            "wo": np.ascontiguousarray(W_o[sl, :]).astype(f16),
            "ident": ident,
        })
    return in_maps


def run(q, k, v, W_q, W_k, W_v, W_o, trace=False):
    n = q.shape[0]
    nc = _get_nc(n=n, din=q.shape[1])
    in_maps = make_in_maps(q, k, v, W_q, W_k, W_v, W_o)
    res = run_bass_kernel_spmd(nc, in_maps, list(range(NCORES)), trace=trace)
    out = res.results[0]["out"].astype(np.float32)
    for c in range(1, NCORES):
        out += res.results[c]["out"]
    return out, res


def kernel(q, k, v, W_q, W_k, W_v, W_o):
    out, _ = run(q, k, v, W_q, W_k, W_v, W_o)
    return out
